# revision 8
# baseline (speedup 1.0000x reference)
"""Causal self-attention (B=2, L=2048, HID=2048, H=16, D=128) on 8 trn2 cores.

Sharding: core c -> (batch b = c//4, head-group g = c%4 of 4 heads).
Each core computes q/k/v projections for its 512 features from its batch,
RoPE, causal attention for its 4 heads, and a partial output projection
against its Wo column slice. Host sums the 4 partials per batch.

Precision (zone-split): sequence positions [0, 512) — whose outputs are
large (few-key softmax rows pass v through) — run fp16 end to end.
Positions [512, 2048) (3/4 of the work, small diffuse outputs) run the
big-contraction matmuls (QKV projections, attn@V, softmax denominator, Wo)
in fp8e4 with perf_mode=DoubleRow — 2 fp8 MACs per PE cell per cycle,
contraction 256 per pass — with fp32 PSUM accumulation. All weights are
pre-scaled x16 on the host (fp8 normal range); the scale is folded into the
exp() argument and the final output copy. The score matmul (contraction
128, no DoubleRow benefit) stays fp16 via fp16 qr/kr. Softmax skips
max-subtraction: exp gets a -3.5 bias that cancels in the normalization
and keeps exp() of any realizable score below fp8e4's 240 max; fp8 e-tile
underflow is harmless because every deep row's denominator keeps the fp16
zone tiles' mass (dn is never ~0), so no clamp ops are needed. The
denominator uses an all-ones stationary matmul so the partition-dim
reduction runs on the PE. Phases (V-proj, QK-proj+RoPE, attention, Wo)
are interleaved per 512-column block to keep the PE saturated.
"""
import numpy as np
import ml_dtypes

import concourse.mybir as mybir
import concourse.tile as tile
from concourse import bacc
from concourse.bass_utils import run_bass_kernel_spmd

B, L, HID, H = 2, 2048, 2048, 16
D = 128               # head dim
NCORES = 8
GH = 4                # heads per core
E = GH * D            # 512 per-core qkv features
NT = HID // 128       # 16 contraction tiles
NI = L // 512         # 4 i-chunks of 512
SCALE = 1.0 / float(np.sqrt(D))

F32 = mybir.dt.float32
MULT = mybir.AluOpType.mult
ADD = mybir.AluOpType.add
IS_GE = mybir.AluOpType.is_ge
F16 = mybir.dt.float16
F8 = mybir.dt.float8e4
DR = mybir.MatmulPerfMode.DoubleRow
EXP = mybir.ActivationFunctionType.Exp
COPY = mybir.ActivationFunctionType.Copy
NP_F16 = np.float16
NP_F8 = ml_dtypes.float8_e4m3

WS = 16.0                   # host-side weight prescale for fp8 range
EXP_BIAS = -3.5             # exp(s*scale - 3.5); cancels in the normalization


def _emit(nc, tc, ctx, io):
    xT, xT16, wqT, wkT, wvT, woT = (
        io["xT"], io["xT16"], io["wqT"], io["wkT"], io["wvT"], io["woT"])
    wqT16, wkT16, wvT16, woT16 = (
        io["wqT16"], io["wkT16"], io["wvT16"], io["woT16"])
    cosT, sinT, rotT, out = io["cosT"], io["sinT"], io["rotT"], io["out"]

    xTr = xT.rearrange("(t p) i -> p t i", p=128)       # [128, 16, 2048] fp8
    xTr16 = xT16.rearrange("(t p) i -> p t i", p=128)   # [128, 16, 512] fp16
    wqTr = wqT.rearrange("(t p) e -> p t e", p=128)     # [128, 16, 512] fp8
    wkTr = wkT.rearrange("(t p) e -> p t e", p=128)
    wvTr = wvT.rearrange("(t p) e -> p t e", p=128)
    woTr = woT.rearrange("(s p) f -> p s f", p=128)     # [128, 4, 2048] fp8
    wqTr16 = wqT16.rearrange("(t p) e -> p t e", p=128)
    wkTr16 = wkT16.rearrange("(t p) e -> p t e", p=128)
    wvTr16 = wvT16.rearrange("(t p) e -> p t e", p=128)
    woTr16 = woT16.rearrange("(s p) f -> p s f", p=128)

    pool = ctx.enter_context(tc.tile_pool(name="main", bufs=1))
    zpool = ctx.enter_context(tc.tile_pool(name="zone", bufs=1))
    xpool = ctx.enter_context(tc.tile_pool(name="xsl", bufs=4))
    work = ctx.enter_context(tc.tile_pool(name="work", bufs=2))
    obpool = ctx.enter_context(tc.tile_pool(name="ob", bufs=3))
    dpool = ctx.enter_context(tc.tile_pool(name="dp", bufs=1))
    # single PSUM pool, exactly 8 banks: mm(4) + acc(2) + dn(2)
    ps = ctx.enter_context(tc.tile_pool(name="ps", bufs=4, space="PSUM"))

    def load_quad(ic, g):
        """mt tiles 4g..4g+3 of xT[:, ic*512:+512] in one DMA (fp8)."""
        xq = xpool.tile([128, 4, 512], F8, tag="xsl", name="xq")
        nc.sync.dma_start(xq[:], xTr[:, 4 * g : 4 * g + 4, ic * 512 : (ic + 1) * 512])
        return xq

    def load_quad16(g):
        """fp16 x tiles for the zone block (positions 0..511)."""
        xq = xpool.tile([128, 4, 512], F16, tag="xsl", name="xq16")
        nc.sync.dma_start(xq[:], xTr16[:, 4 * g : 4 * g + 4, :])
        return xq

    # ---------------- phase A: V projection ------------------------------
    # icj=0 (keys 0..511): fp16; icj 1..3: fp8 DoubleRow.
    wv16 = zpool.tile([128, NT, 512], F16, tag="z1", name="wv16")
    for c in range(4):
        nc.sync.dma_start(wv16[:, 4 * c : 4 * c + 4, :], wvTr16[:, 4 * c : 4 * c + 4, :])
    wv_sb = pool.tile([128, NT, 512], F8, tag="wv")
    for c in range(4):
        nc.sync.dma_start(wv_sb[:, 4 * c : 4 * c + 4, :], wvTr[:, 4 * c : 4 * c + 4, :])
    v16 = pool.tile([128, 4, E], F16, tag="v16", name="v16")    # keys 0..511
    v_all = pool.tile([128, NT, E], F8, tag="vall", name="v_all")  # keys 512+

    ebias = pool.tile([128, 1], F32, tag="ebias")
    nc.gpsimd.memset(ebias[:], EXP_BIAS)
    ones16 = pool.tile([128, 128], F16, tag="ones16")
    nc.gpsimd.memset(ones16[:], 1.0)
    ones2 = pool.tile([128, 2, 128], F8, tag="ones8")
    nc.gpsimd.memset(ones2[:], 1.0)
    rot = pool.tile([128, 128], F16, tag="rot")
    nc.sync.dma_start(rot[:], rotT)

    for icj in range(NI):
        vps = [ps.tile([128, 512], F32, tag="mm", name=f"vp{jt}") for jt in range(4)]
        if icj == 0:
            for mt in range(NT):                 # fp16 zone path
                if mt % 4 == 0:
                    xq = load_quad16(mt // 4)
                for jt in range(4):
                    nc.tensor.matmul(
                        vps[jt][:],
                        xq[:, mt % 4, jt * 128 : (jt + 1) * 128],
                        wv16[:, mt, :],
                        start=(mt == 0),
                        stop=(mt == NT - 1),
                    )
            for jt in range(4):
                if jt % 2 == 0:
                    nc.scalar.copy(v16[:, jt, :], vps[jt][:])
                else:
                    nc.vector.tensor_copy(v16[:, jt, :], vps[jt][:])
        else:
            for u in range(NT // 2):             # fp8 DoubleRow pairs
                if u % 2 == 0:
                    xq = load_quad(icj, u // 2)
                w = 2 * (u % 2)
                for jt in range(4):
                    nc.tensor.matmul(
                        vps[jt][:],
                        xq[:, w : w + 2, jt * 128 : (jt + 1) * 128],
                        wv_sb[:, 2 * u : 2 * u + 2, :],
                        start=(u == 0),
                        stop=(u == NT // 2 - 1),
                        perf_mode=DR,
                    )
            for jt in range(4):
                if jt % 2 == 0:
                    nc.scalar.copy(v_all[:, 4 * icj + jt, :], vps[jt][:])
                else:
                    nc.vector.tensor_copy(v_all[:, 4 * icj + jt, :], vps[jt][:])
        if icj == 0:
            # prefetch q/k weights and rope tables behind phase A's compute
            cos_sb = pool.tile([128, L], F32, tag="cos")
            sin_sb = pool.tile([128, L], F32, tag="sin")
            nc.sync.dma_start(cos_sb[:], cosT)
            nc.sync.dma_start(sin_sb[:], sinT)
            wq_sb = pool.tile([128, NT, 512], F8, tag="wq")
            wk_sb = pool.tile([128, NT, 512], F8, tag="wk")
            wq16 = zpool.tile([128, NT, 512], F16, tag="z2", name="wq16")
            wk16 = zpool.tile([128, NT, 512], F16, tag="z1", name="wk16")
        if icj in (1, 2):
            c0 = 2 * (icj - 1)
            for c in (c0, c0 + 1):
                nc.sync.dma_start(wq_sb[:, 4 * c : 4 * c + 4, :], wqTr[:, 4 * c : 4 * c + 4, :])
                nc.sync.dma_start(wk_sb[:, 4 * c : 4 * c + 4, :], wkTr[:, 4 * c : 4 * c + 4, :])
                nc.sync.dma_start(wq16[:, 4 * c : 4 * c + 4, :], wqTr16[:, 4 * c : 4 * c + 4, :])
                nc.sync.dma_start(wk16[:, 4 * c : 4 * c + 4, :], wkTr16[:, 4 * c : 4 * c + 4, :])

    # ---- phases B/C/D interleaved at the 512-column block level ---------
    qr = [pool.tile([128, L], F16, tag=f"qr{h}", name=f"qr{h}") for h in range(GH)]
    kr = [pool.tile([128, L], F16, tag=f"kr{h}", name=f"kr{h}") for h in range(GH)]
    ot16 = pool.tile([128, GH, 512], F16, tag="ot16", name="ot16")  # queries 0..511
    ot_all = pool.tile([128, GH, L], F8, tag="ot", name="ot_all")   # queries 512+
    e16 = pool.tile([128, 4, 512], F16, tag="e16", name="e16")      # key tiles 0..3
    e_all = pool.tile([128, NT, 512], F8, tag="eall", name="e_all")  # key tiles 4+
    wo_sb = pool.tile([128, GH, L], F8, tag="wo")
    for s_ in range(GH):
        nc.sync.dma_start(wo_sb[:, s_, :], woTr[:, s_, :])

    def emit_rope(batch):
        for pre, dst, dt, isl_ in batch:
            rp = ps.tile([128, 512], F32, tag="acc", bufs=2)
            nc.tensor.matmul(rp[:], rot[:], pre[:], start=True, stop=True)
            t1 = work.tile([128, 512], F32, tag="t1")
            nc.vector.tensor_tensor(t1[:], pre[:], cos_sb[:, isl_], MULT)
            t2 = work.tile([128, 512], F32, tag="t2")
            nc.vector.tensor_tensor(t2[:], rp[:], sin_sb[:, isl_], MULT)
            nc.vector.tensor_tensor(dst[dt][:, isl_], t1[:], t2[:], ADD)

    for ic in range(NI):
        # -- B: q then k projection for this column block --
        isl = slice(ic * 512, (ic + 1) * 512)
        batches = []
        # one set of x quads serves both the q and k groups (bufs=4 holds all)
        quads = [load_quad16(g) if ic == 0 else load_quad(ic, g) for g in range(4)]
        for w_sb16, w_sb8, dst in ((wq16, wq_sb, qr), (wk16, wk_sb, kr)):
            pps = [ps.tile([128, 512], F32, tag="mm", name=f"pp{dt}") for dt in range(GH)]
            if ic == 0:                          # fp16 zone path
                for mt in range(NT):
                    xq = quads[mt // 4]
                    for dt in range(GH):
                        nc.tensor.matmul(
                            pps[dt][:],
                            w_sb16[:, mt, dt * 128 : (dt + 1) * 128],
                            xq[:, mt % 4, :],
                            start=(mt == 0),
                            stop=(mt == NT - 1),
                        )
            else:                                # fp8 DoubleRow path
                for u in range(NT // 2):
                    xq = quads[u // 2]
                    w = 2 * (u % 2)
                    for dt in range(GH):
                        nc.tensor.matmul(
                            pps[dt][:],
                            w_sb8[:, 2 * u : 2 * u + 2, dt * 128 : (dt + 1) * 128],
                            xq[:, w : w + 2, :],
                            start=(u == 0),
                            stop=(u == NT // 2 - 1),
                            perf_mode=DR,
                        )
            batch = []
            for dt in range(GH):
                pre = work.tile([128, 512], F16, tag="pre", bufs=4)
                if dt % 2 == 0:
                    nc.scalar.copy(pre[:], pps[dt][:])
                else:
                    nc.vector.tensor_copy(pre[:], pps[dt][:])
                batch.append((pre, dst, dt, isl))
            batches.append(batch)
            if len(batches) == 2:
                emit_rope(batches[0])  # q rope: its pre tiles finished during k group
        emit_rope(batches[1])
        if ic == 0:
            # zone Wo weights reuse wq16's slot (dead after B-ic0's q group)
            wo16 = zpool.tile([128, GH, L // 2], F16, tag="z2", name="wo16")
            for s_ in range(GH):
                nc.sync.dma_start(wo16[:, s_, :], woTr16[:, s_, : L // 2])
            wo16b = zpool.tile([128, GH, L // 2], F16, tag="z1", name="wo16b")
            for s_ in range(GH):
                nc.sync.dma_start(wo16b[:, s_, :], woTr16[:, s_, L // 2 :])

        # -- C: attention for query block I = ic, all heads --
        I = ic
        nj = (I + 1) * 4
        i0 = I * 512

        def vc0(jt):
            # diag tile jt = I*4 + t has valid columns [128*t, 512) only
            return max(0, (jt - I * 4) * 128)

        def vc0p(jt):
            # pair-granular start for DoubleRow pairs (2u, 2u+1)
            return vc0(jt - (jt - I * 4) % 2) if jt >= I * 4 else 0

        for h in range(GH):
            for jt in range(nj):
                c0_ = vc0(jt)
                zone = jt < 4
                cp_ = c0_ if zone else vc0p(jt)
                st = ps.tile([128, 512], F32, tag="mm", name="st")
                nc.tensor.matmul(
                    st[:, c0_:],
                    kr[h][:, jt * 128 : (jt + 1) * 128],
                    qr[h][:, i0 + c0_ : i0 + 512],
                    start=True,
                    stop=True,
                )
                et = e16[:, jt, :] if zone else e_all[:, jt, :]
                nc.scalar.activation(
                    et[:, cp_:], st[:, cp_:], EXP,
                    scale=SCALE / (WS * WS), bias=ebias[:],
                )
                if jt >= I * 4:
                    # within valid cols keep upper triangle: c' - p >= base.
                    # No fp8 cap/floor needed: bias -3.5 puts exp() overflow
                    # beyond any realizable score, and deep rows always have
                    # the fp16 zone tiles' denominator mass (dn never ~0);
                    # deep-tile underflow only drops <1e-4-mass weights.
                    nc.gpsimd.affine_select(
                        out=et[:, cp_:],
                        in_=et[:, cp_:],
                        compare_op=IS_GE,
                        fill=0.0,
                        base=cp_ - c0_,
                        pattern=[[1, 512 - cp_]],
                        channel_multiplier=-1,
                    )
            ovtag, dntag = ("dn", "acc") if h % 2 == 0 else ("acc", "dn")
            ov = ps.tile([128, 512], F32, tag=ovtag, bufs=2)
            nzone = min(nj, 4)
            for jt in range(nzone):              # zone: fp16 key tiles 0..3
                c0_ = vc0(jt)
                nc.tensor.matmul(
                    ov[:, c0_:],
                    v16[:, jt, h * 128 : (h + 1) * 128],
                    e16[:, jt, c0_:],
                    start=(jt == 0),
                    stop=(jt == nzone - 1 and nj == nzone),
                )
            for u in range(2, nj // 2):          # deep: fp8 DoubleRow pairs
                cp_ = vc0p(2 * u)
                nc.tensor.matmul(
                    ov[:, cp_:],
                    v_all[:, 2 * u : 2 * u + 2, h * 128 : (h + 1) * 128],
                    e_all[:, 2 * u : 2 * u + 2, cp_:],
                    start=False,
                    stop=(u == nj // 2 - 1),
                    perf_mode=DR,
                )
            dn = ps.tile([128, 512], F32, tag=dntag, bufs=2)
            for jt in range(nzone):
                c0_ = vc0(jt)
                nc.tensor.matmul(
                    dn[:, c0_:], ones16[:], e16[:, jt, c0_:],
                    start=(jt == 0), stop=(jt == nzone - 1 and nj == nzone),
                )
            for u in range(2, nj // 2):
                cp_ = vc0p(2 * u)
                nc.tensor.matmul(
                    dn[:, cp_:], ones2[:], e_all[:, 2 * u : 2 * u + 2, cp_:],
                    start=False, stop=(u == nj // 2 - 1),
                    perf_mode=DR,
                )
            rbi = dpool.tile([128, 512], F32, tag="rbi", bufs=2)
            nc.vector.reciprocal_approx_fast(out=rbi[:], in_=dn[:])
            if ic == 0:
                nc.vector.tensor_tensor(ot16[:, h, :], ov[:], rbi[:], MULT)
            else:
                # fp8 DVE writes are slow: multiply into fp16, cast on ScalarE
                otw = work.tile([128, 512], F16, tag="otw", bufs=2)
                nc.vector.tensor_tensor(otw[:], ov[:], rbi[:], MULT)
                nc.scalar.copy(ot_all[:, h, i0 : i0 + 512], otw[:])

        # -- D: Wo blocks for query tiles completed by this block --
        dtags = [("mm", 4), ("mm", 4), ("acc", 2), ("dn", 2)]
        for it in range(I * 4, I * 4 + 4):
            for fp in range(2):  # fc pairs
                ob = obpool.tile([128, 1024], F16, tag="ob", bufs=2)
                for half in range(2):
                    fc = 2 * fp + half
                    dtag, dbufs = dtags[(it * NI + fc) % 4]
                    op = ps.tile([128, 512], F32, tag=dtag, bufs=dbufs, name="op")
                    if ic == 0:                  # fp16 zone path (queries 0..511)
                        w16 = wo16 if fc < 2 else wo16b
                        fo = fc % 2
                        for h in range(GH):
                            nc.tensor.matmul(
                                op[:],
                                ot16[:, h, it * 128 : (it + 1) * 128],
                                w16[:, h, fo * 512 : (fo + 1) * 512],
                                start=(h == 0),
                                stop=(h == GH - 1),
                            )
                    else:                        # fp8 DoubleRow path
                        for u in range(GH // 2):
                            nc.tensor.matmul(
                                op[:],
                                ot_all[:, 2 * u : 2 * u + 2, it * 128 : (it + 1) * 128],
                                wo_sb[:, 2 * u : 2 * u + 2, fc * 512 : (fc + 1) * 512],
                                start=(u == 0),
                                stop=(u == GH // 2 - 1),
                                perf_mode=DR,
                            )
                    osl = ob[:, half * 512 : (half + 1) * 512]
                    if (it + fc) % 2 == 0:
                        nc.vector.tensor_scalar_mul(osl, op[:], 1.0 / (WS * WS))
                    else:
                        nc.scalar.activation(osl, op[:], COPY, scale=1.0 / (WS * WS))
                nc.sync.dma_start(
                    out[it * 128 : (it + 1) * 128, fp * 1024 : (fp + 1) * 1024], ob[:]
                )


def build():
    import contextlib

    nc = bacc.Bacc("TRN2", target_bir_lowering=False, debug=False, num_devices=NCORES)
    io = {
        "xT": nc.dram_tensor("xT", [HID, L], F8, kind="ExternalInput").ap(),
        "xT16": nc.dram_tensor("xT16", [HID, 512], F16, kind="ExternalInput").ap(),
        "wqT": nc.dram_tensor("wqT", [HID, E], F8, kind="ExternalInput").ap(),
        "wkT": nc.dram_tensor("wkT", [HID, E], F8, kind="ExternalInput").ap(),
        "wvT": nc.dram_tensor("wvT", [HID, E], F8, kind="ExternalInput").ap(),
        "woT": nc.dram_tensor("woT", [E, HID], F8, kind="ExternalInput").ap(),
        "wqT16": nc.dram_tensor("wqT16", [HID, E], F16, kind="ExternalInput").ap(),
        "wkT16": nc.dram_tensor("wkT16", [HID, E], F16, kind="ExternalInput").ap(),
        "wvT16": nc.dram_tensor("wvT16", [HID, E], F16, kind="ExternalInput").ap(),
        "woT16": nc.dram_tensor("woT16", [E, HID], F16, kind="ExternalInput").ap(),
        "cosT": nc.dram_tensor("cosT", [D, L], F32, kind="ExternalInput").ap(),
        "sinT": nc.dram_tensor("sinT", [D, L], F32, kind="ExternalInput").ap(),
        "rotT": nc.dram_tensor("rotT", [D, D], F16, kind="ExternalInput").ap(),
        "out": nc.dram_tensor("out", [L, HID], F16, kind="ExternalOutput").ap(),
    }
    with tile.TileContext(nc) as tc:
        with contextlib.ExitStack() as ctx:
            _emit(nc, tc, ctx, io)
    nc.compile()
    return nc


_NC_CACHE = []


def _rot_matrix():
    # lhsT for the rotate_half matmul: rot(q) = P @ q, lhsT = P^T.
    rotT = np.zeros((D, D), dtype=NP_F16)
    for d in range(D // 2):
        rotT[d, d + 64] = 1.0
        rotT[d + 64, d] = -1.0
    return rotT


def _to8(a):
    return np.clip(a, -240.0, 240.0).astype(NP_F8)


def make_in_maps(hidden_states, cos, sin, Wq, Wk, Wv, Wo):
    cosT = np.ascontiguousarray(cos.T.astype(np.float32))
    sinT = np.ascontiguousarray(sin.T.astype(np.float32))
    rotT = _rot_matrix()
    xTs = [np.ascontiguousarray(hidden_states[b].T.astype(np.float32)) for b in range(B)]
    in_maps = []
    for c in range(NCORES):
        b, g = divmod(c, 4)
        sl = slice(g * E, (g + 1) * E)
        wq = np.ascontiguousarray(Wq[sl].T) * WS
        wk = np.ascontiguousarray(Wk[sl].T) * WS
        wv = np.ascontiguousarray(Wv[sl].T) * WS
        wo = np.ascontiguousarray(Wo[:, sl].T) * WS
        in_maps.append({
            "xT": _to8(xTs[b]),
            "xT16": np.ascontiguousarray(xTs[b][:, :512]).astype(NP_F16),
            "wqT": _to8(wq), "wkT": _to8(wk), "wvT": _to8(wv), "woT": _to8(wo),
            "wqT16": wq.astype(NP_F16), "wkT16": wk.astype(NP_F16),
            "wvT16": wv.astype(NP_F16), "woT16": wo.astype(NP_F16),
            "cosT": cosT,
            "sinT": sinT,
            "rotT": rotT,
        })
    return in_maps


def kernel(hidden_states, cos, sin, Wq, Wk, Wv, Wo):
    hidden_states, cos, sin, Wq, Wk, Wv, Wo = (
        np.asarray(a) for a in (hidden_states, cos, sin, Wq, Wk, Wv, Wo)
    )
    if not _NC_CACHE:
        _NC_CACHE.append(build())
    nc = _NC_CACHE[0]
    in_maps = make_in_maps(hidden_states, cos, sin, Wq, Wk, Wv, Wo)
    r = run_bass_kernel_spmd(nc, in_maps, list(range(NCORES)))
    out = np.empty((B, L, HID), np.float32)
    for b in range(B):
        acc = r.results[4 * b]["out"].astype(np.float32)
        for g in range(1, 4):
            acc += r.results[4 * b + g]["out"].astype(np.float32)
        out[b] = acc
    return out


# revision 9
# speedup vs baseline: 1.0333x; 1.0333x over previous
"""Causal self-attention (B=2, L=2048, HID=2048, H=16, D=128) on 8 trn2 cores.

Sharding: core c -> (batch b = c//4, head-group g = c%4 of 4 heads).
Each core computes q/k/v projections for its 512 features from its batch,
RoPE, causal attention for its 4 heads, and a partial output projection
against its Wo column slice. Host sums the 4 partials per batch.

Precision (zone-split): sequence positions [0, 512) — whose outputs are
large (few-key softmax rows pass v through) — run fp16 end to end.
Positions [512, 2048) (3/4 of the work, small diffuse outputs) run the
big-contraction matmuls (QKV projections, attn@V, softmax denominator, Wo)
in fp8e4 with perf_mode=DoubleRow — 2 fp8 MACs per PE cell per cycle,
contraction 256 per pass — with fp32 PSUM accumulation. All weights are
pre-scaled x16 on the host (fp8 normal range); the scale is folded into the
exp() argument and the final output copy. The score matmul (contraction
128, no DoubleRow benefit) stays fp16 via fp16 qr/kr. Softmax skips
max-subtraction: exp gets a -3.5 bias that cancels in the normalization
and keeps exp() of any realizable score below fp8e4's 240 max; fp8 e-tile
underflow is harmless because every deep row's denominator keeps the fp16
zone tiles' mass (dn is never ~0), so no clamp ops are needed. The
denominator uses an all-ones stationary matmul so the partition-dim
reduction runs on the PE. Phases (V-proj, QK-proj+RoPE, attention, Wo)
are interleaved per 512-column block to keep the PE saturated.
"""
import numpy as np
import ml_dtypes

import concourse.mybir as mybir
import concourse.tile as tile
from concourse import bacc
from concourse.bass_utils import run_bass_kernel_spmd

B, L, HID, H = 2, 2048, 2048, 16
D = 128               # head dim
NCORES = 8
GH = 4                # heads per core
E = GH * D            # 512 per-core qkv features
NT = HID // 128       # 16 contraction tiles
NI = L // 512         # 4 i-chunks of 512
SCALE = 1.0 / float(np.sqrt(D))

F32 = mybir.dt.float32
MULT = mybir.AluOpType.mult
ADD = mybir.AluOpType.add
IS_GE = mybir.AluOpType.is_ge
F16 = mybir.dt.float16
F8 = mybir.dt.float8e4
DR = mybir.MatmulPerfMode.DoubleRow
EXP = mybir.ActivationFunctionType.Exp
COPY = mybir.ActivationFunctionType.Copy
NP_F16 = np.float16
NP_F8 = ml_dtypes.float8_e4m3

WS = 16.0                   # host-side weight prescale for fp8 range
EXP_BIAS = -3.5             # exp(s*scale - 3.5); cancels in the normalization


def _emit(nc, tc, ctx, io):
    xT, xT16, wqT, wkT, wvT, woT = (
        io["xT"], io["xT16"], io["wqT"], io["wkT"], io["wvT"], io["woT"])
    wqT16, wkT16, wvT16, woT16 = (
        io["wqT16"], io["wkT16"], io["wvT16"], io["woT16"])
    cosT, sinT, rotT, out = io["cosT"], io["sinT"], io["rotT"], io["out"]

    xTr = xT.rearrange("(t p) i -> p t i", p=128)       # [128, 16, 2048] fp8
    xTr16 = xT16.rearrange("(t p) i -> p t i", p=128)   # [128, 16, 512] fp16
    wqTr = wqT.rearrange("(t p) e -> p t e", p=128)     # [128, 16, 512] fp8
    wkTr = wkT.rearrange("(t p) e -> p t e", p=128)
    wvTr = wvT.rearrange("(t p) e -> p t e", p=128)
    woTr = woT.rearrange("(s p) f -> p s f", p=128)     # [128, 4, 2048] fp8
    wqTr16 = wqT16.rearrange("(t p) e -> p t e", p=128)
    wkTr16 = wkT16.rearrange("(t p) e -> p t e", p=128)
    wvTr16 = wvT16.rearrange("(t p) e -> p t e", p=128)
    woTr16 = woT16.rearrange("(s p) f -> p s f", p=128)

    pool = ctx.enter_context(tc.tile_pool(name="main", bufs=1))
    zpool = ctx.enter_context(tc.tile_pool(name="zone", bufs=1))
    xpool = ctx.enter_context(tc.tile_pool(name="xsl", bufs=4))
    work = ctx.enter_context(tc.tile_pool(name="work", bufs=2))
    obpool = ctx.enter_context(tc.tile_pool(name="ob", bufs=3))
    dpool = ctx.enter_context(tc.tile_pool(name="dp", bufs=1))
    # single PSUM pool, exactly 8 banks: mm(4) + acc(2) + dn(2)
    ps = ctx.enter_context(tc.tile_pool(name="ps", bufs=4, space="PSUM"))

    def load_quad(ic, g):
        """mt tiles 4g..4g+3 of xT[:, ic*512:+512] in one DMA (fp8)."""
        xq = xpool.tile([128, 4, 512], F8, tag="xsl", name="xq")
        nc.sync.dma_start(xq[:], xTr[:, 4 * g : 4 * g + 4, ic * 512 : (ic + 1) * 512])
        return xq

    def load_quad16(g):
        """fp16 x tiles for the zone block (positions 0..511)."""
        xq = xpool.tile([128, 4, 512], F16, tag="xsl", name="xq16")
        nc.sync.dma_start(xq[:], xTr16[:, 4 * g : 4 * g + 4, :])
        return xq

    # ---------------- phase A: V projection ------------------------------
    # icj=0 (keys 0..511): fp16; icj 1..3: fp8 DoubleRow.
    wv16 = zpool.tile([128, NT, 512], F16, tag="z1", name="wv16")
    for c in range(4):
        nc.sync.dma_start(wv16[:, 4 * c : 4 * c + 4, :], wvTr16[:, 4 * c : 4 * c + 4, :])
    wv_sb = pool.tile([128, NT, 512], F8, tag="wv")
    for c in range(4):
        nc.sync.dma_start(wv_sb[:, 4 * c : 4 * c + 4, :], wvTr[:, 4 * c : 4 * c + 4, :])
    v16 = pool.tile([128, 4, E], F16, tag="v16", name="v16")    # keys 0..511
    v_all = pool.tile([128, NT, E], F8, tag="vall", name="v_all")  # keys 512+

    ebias = pool.tile([128, 1], F32, tag="ebias")
    nc.gpsimd.memset(ebias[:], EXP_BIAS)
    ones16 = pool.tile([128, 128], F16, tag="ones16")
    nc.gpsimd.memset(ones16[:], 1.0)
    ones2 = pool.tile([128, 2, 128], F8, tag="ones8")
    nc.gpsimd.memset(ones2[:], 1.0)
    rot = pool.tile([128, 128], F16, tag="rot")
    nc.sync.dma_start(rot[:], rotT)

    for icj in range(NI):
        vps = [ps.tile([128, 512], F32, tag="mm", name=f"vp{jt}") for jt in range(4)]
        if icj == 0:
            for mt in range(NT):                 # fp16 zone path
                if mt % 4 == 0:
                    xq = load_quad16(mt // 4)
                for jt in range(4):
                    nc.tensor.matmul(
                        vps[jt][:],
                        xq[:, mt % 4, jt * 128 : (jt + 1) * 128],
                        wv16[:, mt, :],
                        start=(mt == 0),
                        stop=(mt == NT - 1),
                    )
            for jt in range(4):
                if jt % 2 == 0:
                    nc.scalar.copy(v16[:, jt, :], vps[jt][:])
                else:
                    nc.vector.tensor_copy(v16[:, jt, :], vps[jt][:])
        else:
            for u in range(NT // 2):             # fp8 DoubleRow pairs
                if u % 2 == 0:
                    xq = load_quad(icj, u // 2)
                w = 2 * (u % 2)
                for jt in range(4):
                    nc.tensor.matmul(
                        vps[jt][:],
                        xq[:, w : w + 2, jt * 128 : (jt + 1) * 128],
                        wv_sb[:, 2 * u : 2 * u + 2, :],
                        start=(u == 0),
                        stop=(u == NT // 2 - 1),
                        perf_mode=DR,
                    )
            for jt in range(4):
                if jt % 2 == 0:
                    nc.scalar.copy(v_all[:, 4 * icj + jt, :], vps[jt][:])
                else:
                    nc.vector.tensor_copy(v_all[:, 4 * icj + jt, :], vps[jt][:])
        if icj == 0:
            # prefetch q/k weights and rope tables behind phase A's compute
            cos_sb = pool.tile([128, L], F32, tag="cos")
            sin_sb = pool.tile([128, L], F32, tag="sin")
            nc.sync.dma_start(cos_sb[:], cosT)
            nc.sync.dma_start(sin_sb[:], sinT)
            wq_sb = pool.tile([128, NT, 512], F8, tag="wq")
            wk_sb = pool.tile([128, NT, 512], F8, tag="wk")
            wq16 = zpool.tile([128, NT, 512], F16, tag="z2", name="wq16")
            wk16 = zpool.tile([128, NT, 512], F16, tag="z1", name="wk16")
        if icj in (1, 2):
            c0 = 2 * (icj - 1)
            for c in (c0, c0 + 1):
                nc.sync.dma_start(wq_sb[:, 4 * c : 4 * c + 4, :], wqTr[:, 4 * c : 4 * c + 4, :])
                nc.sync.dma_start(wk_sb[:, 4 * c : 4 * c + 4, :], wkTr[:, 4 * c : 4 * c + 4, :])
                nc.sync.dma_start(wq16[:, 4 * c : 4 * c + 4, :], wqTr16[:, 4 * c : 4 * c + 4, :])
                nc.sync.dma_start(wk16[:, 4 * c : 4 * c + 4, :], wkTr16[:, 4 * c : 4 * c + 4, :])

    # ---- phases B/C/D interleaved at the 512-column block level ---------
    qr = [pool.tile([128, L], F16, tag=f"qr{h}", name=f"qr{h}") for h in range(GH)]
    kr = [pool.tile([128, L], F16, tag=f"kr{h}", name=f"kr{h}") for h in range(GH)]
    ot16 = pool.tile([128, GH, 512], F16, tag="ot16", name="ot16")  # queries 0..511
    ot_all = pool.tile([128, GH, L], F8, tag="ot", name="ot_all")   # queries 512+
    e16 = pool.tile([128, 4, 512], F16, tag="e16", name="e16")      # key tiles 0..3
    e_all = pool.tile([128, NT, 512], F8, tag="eall", name="e_all")  # key tiles 4+
    wo_sb = pool.tile([128, GH, L], F8, tag="wo")
    for s_ in range(GH):
        nc.sync.dma_start(wo_sb[:, s_, :], woTr[:, s_, :])

    def emit_rope(batch):
        for pre, dst, dt, isl_ in batch:
            rp = ps.tile([128, 512], F32, tag="acc", bufs=2)
            nc.tensor.matmul(rp[:], rot[:], pre[:], start=True, stop=True)
            t1 = work.tile([128, 512], F32, tag="t1")
            nc.vector.tensor_tensor(t1[:], pre[:], cos_sb[:, isl_], MULT)
            t2 = work.tile([128, 512], F32, tag="t2")
            nc.vector.tensor_tensor(t2[:], rp[:], sin_sb[:, isl_], MULT)
            nc.vector.tensor_tensor(dst[dt][:, isl_], t1[:], t2[:], ADD)

    for ic in range(NI):
        # -- B: q then k projection for this column block --
        isl = slice(ic * 512, (ic + 1) * 512)
        batches = []
        for w_sb16, w_sb8, dst in ((wq16, wq_sb, qr), (wk16, wk_sb, kr)):
            pps = [ps.tile([128, 512], F32, tag="mm", name=f"pp{dt}") for dt in range(GH)]
            if ic == 0:                          # fp16 zone path
                for mt in range(NT):
                    if mt % 4 == 0:
                        xq = load_quad16(mt // 4)
                    for dt in range(GH):
                        nc.tensor.matmul(
                            pps[dt][:],
                            w_sb16[:, mt, dt * 128 : (dt + 1) * 128],
                            xq[:, mt % 4, :],
                            start=(mt == 0),
                            stop=(mt == NT - 1),
                        )
            else:                                # fp8 DoubleRow path
                for u in range(NT // 2):
                    if u % 2 == 0:
                        xq = load_quad(ic, u // 2)
                    w = 2 * (u % 2)
                    for dt in range(GH):
                        nc.tensor.matmul(
                            pps[dt][:],
                            w_sb8[:, 2 * u : 2 * u + 2, dt * 128 : (dt + 1) * 128],
                            xq[:, w : w + 2, :],
                            start=(u == 0),
                            stop=(u == NT // 2 - 1),
                            perf_mode=DR,
                        )
            batch = []
            for dt in range(GH):
                pre = work.tile([128, 512], F16, tag="pre", bufs=4)
                if dt % 2 == 0:
                    nc.scalar.copy(pre[:], pps[dt][:])
                else:
                    nc.vector.tensor_copy(pre[:], pps[dt][:])
                batch.append((pre, dst, dt, isl))
            batches.append(batch)
            if len(batches) == 2:
                emit_rope(batches[0])  # q rope: its pre tiles finished during k group
        emit_rope(batches[1])
        if ic == 0:
            # zone Wo weights reuse wq16's slot (dead after B-ic0's q group)
            wo16 = zpool.tile([128, GH, L // 2], F16, tag="z2", name="wo16")
            for s_ in range(GH):
                nc.sync.dma_start(wo16[:, s_, :], woTr16[:, s_, : L // 2])
            wo16b = zpool.tile([128, GH, L // 2], F16, tag="z1", name="wo16b")
            for s_ in range(GH):
                nc.sync.dma_start(wo16b[:, s_, :], woTr16[:, s_, L // 2 :])

        # -- C: attention for query block I = ic, all heads --
        I = ic
        nj = (I + 1) * 4
        i0 = I * 512

        def vc0(jt):
            # diag tile jt = I*4 + t has valid columns [128*t, 512) only
            return max(0, (jt - I * 4) * 128)

        def vc0p(jt):
            # pair-granular start for DoubleRow pairs (2u, 2u+1)
            return vc0(jt - (jt - I * 4) % 2) if jt >= I * 4 else 0

        for h in range(GH):
            for jt in range(nj):
                c0_ = vc0(jt)
                zone = jt < 4
                cp_ = c0_ if zone else vc0p(jt)
                st = ps.tile([128, 512], F32, tag="mm", name="st")
                nc.tensor.matmul(
                    st[:, c0_:],
                    kr[h][:, jt * 128 : (jt + 1) * 128],
                    qr[h][:, i0 + c0_ : i0 + 512],
                    start=True,
                    stop=True,
                )
                et = e16[:, jt, :] if zone else e_all[:, jt, :]
                nc.scalar.activation(
                    et[:, cp_:], st[:, cp_:], EXP,
                    scale=SCALE / (WS * WS), bias=ebias[:],
                )
                if jt >= I * 4:
                    # within valid cols keep upper triangle: c' - p >= base.
                    # No fp8 cap/floor needed: bias -3.5 puts exp() overflow
                    # beyond any realizable score, and deep rows always have
                    # the fp16 zone tiles' denominator mass (dn never ~0);
                    # deep-tile underflow only drops <1e-4-mass weights.
                    nc.gpsimd.affine_select(
                        out=et[:, cp_:],
                        in_=et[:, cp_:],
                        compare_op=IS_GE,
                        fill=0.0,
                        base=cp_ - c0_,
                        pattern=[[1, 512 - cp_]],
                        channel_multiplier=-1,
                    )
            ovtag, dntag = ("dn", "acc") if h % 2 == 0 else ("acc", "dn")
            ov = ps.tile([128, 512], F32, tag=ovtag, bufs=2)
            nzone = min(nj, 4)
            for jt in range(nzone):              # zone: fp16 key tiles 0..3
                c0_ = vc0(jt)
                nc.tensor.matmul(
                    ov[:, c0_:],
                    v16[:, jt, h * 128 : (h + 1) * 128],
                    e16[:, jt, c0_:],
                    start=(jt == 0),
                    stop=(jt == nzone - 1 and nj == nzone),
                )
            for u in range(2, nj // 2):          # deep: fp8 DoubleRow pairs
                cp_ = vc0p(2 * u)
                nc.tensor.matmul(
                    ov[:, cp_:],
                    v_all[:, 2 * u : 2 * u + 2, h * 128 : (h + 1) * 128],
                    e_all[:, 2 * u : 2 * u + 2, cp_:],
                    start=False,
                    stop=(u == nj // 2 - 1),
                    perf_mode=DR,
                )
            dn = ps.tile([128, 512], F32, tag=dntag, bufs=2)
            for jt in range(nzone):
                c0_ = vc0(jt)
                nc.tensor.matmul(
                    dn[:, c0_:], ones16[:], e16[:, jt, c0_:],
                    start=(jt == 0), stop=(jt == nzone - 1 and nj == nzone),
                )
            for u in range(2, nj // 2):
                cp_ = vc0p(2 * u)
                nc.tensor.matmul(
                    dn[:, cp_:], ones2[:], e_all[:, 2 * u : 2 * u + 2, cp_:],
                    start=False, stop=(u == nj // 2 - 1),
                    perf_mode=DR,
                )
            rbi = dpool.tile([128, 512], F32, tag="rbi", bufs=2)
            nc.vector.reciprocal_approx_fast(out=rbi[:], in_=dn[:])
            if ic == 0:
                nc.vector.tensor_tensor(ot16[:, h, :], ov[:], rbi[:], MULT)
            else:
                # fp8 DVE writes are slow: multiply into fp16, cast on ScalarE
                otw = work.tile([128, 512], F16, tag="otw", bufs=2)
                nc.vector.tensor_tensor(otw[:], ov[:], rbi[:], MULT)
                nc.scalar.copy(ot_all[:, h, i0 : i0 + 512], otw[:])

        # -- D: Wo blocks for query tiles completed by this block --
        dtags = [("mm", 4), ("mm", 4), ("acc", 2), ("dn", 2)]
        for it in range(I * 4, I * 4 + 4):
            for fp in range(2):  # fc pairs
                ob = obpool.tile([128, 1024], F16, tag="ob", bufs=2)
                for half in range(2):
                    fc = 2 * fp + half
                    dtag, dbufs = dtags[(it * NI + fc) % 4]
                    op = ps.tile([128, 512], F32, tag=dtag, bufs=dbufs, name="op")
                    if ic == 0:                  # fp16 zone path (queries 0..511)
                        w16 = wo16 if fc < 2 else wo16b
                        fo = fc % 2
                        for h in range(GH):
                            nc.tensor.matmul(
                                op[:],
                                ot16[:, h, it * 128 : (it + 1) * 128],
                                w16[:, h, fo * 512 : (fo + 1) * 512],
                                start=(h == 0),
                                stop=(h == GH - 1),
                            )
                    else:                        # fp8 DoubleRow path
                        for u in range(GH // 2):
                            nc.tensor.matmul(
                                op[:],
                                ot_all[:, 2 * u : 2 * u + 2, it * 128 : (it + 1) * 128],
                                wo_sb[:, 2 * u : 2 * u + 2, fc * 512 : (fc + 1) * 512],
                                start=(u == 0),
                                stop=(u == GH // 2 - 1),
                                perf_mode=DR,
                            )
                    osl = ob[:, half * 512 : (half + 1) * 512]
                    if (it + fc) % 2 == 0:
                        nc.vector.tensor_scalar_mul(osl, op[:], 1.0 / (WS * WS))
                    else:
                        nc.scalar.activation(osl, op[:], COPY, scale=1.0 / (WS * WS))
                nc.sync.dma_start(
                    out[it * 128 : (it + 1) * 128, fp * 1024 : (fp + 1) * 1024], ob[:]
                )


def build():
    import contextlib

    nc = bacc.Bacc("TRN2", target_bir_lowering=False, debug=False, num_devices=NCORES)
    io = {
        "xT": nc.dram_tensor("xT", [HID, L], F8, kind="ExternalInput").ap(),
        "xT16": nc.dram_tensor("xT16", [HID, 512], F16, kind="ExternalInput").ap(),
        "wqT": nc.dram_tensor("wqT", [HID, E], F8, kind="ExternalInput").ap(),
        "wkT": nc.dram_tensor("wkT", [HID, E], F8, kind="ExternalInput").ap(),
        "wvT": nc.dram_tensor("wvT", [HID, E], F8, kind="ExternalInput").ap(),
        "woT": nc.dram_tensor("woT", [E, HID], F8, kind="ExternalInput").ap(),
        "wqT16": nc.dram_tensor("wqT16", [HID, E], F16, kind="ExternalInput").ap(),
        "wkT16": nc.dram_tensor("wkT16", [HID, E], F16, kind="ExternalInput").ap(),
        "wvT16": nc.dram_tensor("wvT16", [HID, E], F16, kind="ExternalInput").ap(),
        "woT16": nc.dram_tensor("woT16", [E, HID], F16, kind="ExternalInput").ap(),
        "cosT": nc.dram_tensor("cosT", [D, L], F32, kind="ExternalInput").ap(),
        "sinT": nc.dram_tensor("sinT", [D, L], F32, kind="ExternalInput").ap(),
        "rotT": nc.dram_tensor("rotT", [D, D], F16, kind="ExternalInput").ap(),
        "out": nc.dram_tensor("out", [L, HID], F16, kind="ExternalOutput").ap(),
    }
    with tile.TileContext(nc) as tc:
        with contextlib.ExitStack() as ctx:
            _emit(nc, tc, ctx, io)
    nc.compile()
    return nc


_NC_CACHE = []


def _rot_matrix():
    # lhsT for the rotate_half matmul: rot(q) = P @ q, lhsT = P^T.
    rotT = np.zeros((D, D), dtype=NP_F16)
    for d in range(D // 2):
        rotT[d, d + 64] = 1.0
        rotT[d + 64, d] = -1.0
    return rotT


def _to8(a):
    return np.clip(a, -240.0, 240.0).astype(NP_F8)


def make_in_maps(hidden_states, cos, sin, Wq, Wk, Wv, Wo):
    cosT = np.ascontiguousarray(cos.T.astype(np.float32))
    sinT = np.ascontiguousarray(sin.T.astype(np.float32))
    rotT = _rot_matrix()
    xTs = [np.ascontiguousarray(hidden_states[b].T.astype(np.float32)) for b in range(B)]
    in_maps = []
    for c in range(NCORES):
        b, g = divmod(c, 4)
        sl = slice(g * E, (g + 1) * E)
        wq = np.ascontiguousarray(Wq[sl].T) * WS
        wk = np.ascontiguousarray(Wk[sl].T) * WS
        wv = np.ascontiguousarray(Wv[sl].T) * WS
        wo = np.ascontiguousarray(Wo[:, sl].T) * WS
        in_maps.append({
            "xT": _to8(xTs[b]),
            "xT16": np.ascontiguousarray(xTs[b][:, :512]).astype(NP_F16),
            "wqT": _to8(wq), "wkT": _to8(wk), "wvT": _to8(wv), "woT": _to8(wo),
            "wqT16": wq.astype(NP_F16), "wkT16": wk.astype(NP_F16),
            "wvT16": wv.astype(NP_F16), "woT16": wo.astype(NP_F16),
            "cosT": cosT,
            "sinT": sinT,
            "rotT": rotT,
        })
    return in_maps


def kernel(hidden_states, cos, sin, Wq, Wk, Wv, Wo):
    hidden_states, cos, sin, Wq, Wk, Wv, Wo = (
        np.asarray(a) for a in (hidden_states, cos, sin, Wq, Wk, Wv, Wo)
    )
    if not _NC_CACHE:
        _NC_CACHE.append(build())
    nc = _NC_CACHE[0]
    in_maps = make_in_maps(hidden_states, cos, sin, Wq, Wk, Wv, Wo)
    r = run_bass_kernel_spmd(nc, in_maps, list(range(NCORES)))
    out = np.empty((B, L, HID), np.float32)
    for b in range(B):
        acc = r.results[4 * b]["out"].astype(np.float32)
        for g in range(1, 4):
            acc += r.results[4 * b + g]["out"].astype(np.float32)
        out[b] = acc
    return out
